# revision 17
# baseline (speedup 1.0000x reference)
"""GCN layer (out = A @ x @ W, A sparse COO) on 8 Trainium2 NeuronCores.

Strategy (1D dest partitioning, x replicated):
  - Destinations (output rows) are sharded across the 8 cores; x is
    replicated to every core's HBM, the [64,64] weight is replicated.
  - Host-side preprocessing is pure indexing: dests are PERMUTED into
    64-lane half-blocks via a degree-balanced first-fit assignment so
    that every (half-block, source-chunk) bucket has <= 256 edges, i.e.
    exactly 2 slots of 128 edges.  This removes nearly all slot padding
    (padding directly multiplies gather-DMA time, the bottleneck), and
    the 64-wide dest one-hot halves the DVE work per edge vs 128-wide.
  - Device per core: for each window of dest blocks, for each of the 4
    source chunks: one dma_gather call per segment (descriptor-gen fixed
    cost amortized; calls round-robin the 4 SWDGE queues); DVE builds a
    64-wide one-hot per slot (is_equal vs iota, batched 16 slots per op)
    and scales gathered rows by edge_val (f32->bf16); PE matmuls (bf16)
    accumulate aggT[64 feat, 64 dest] per (block, half) into half-bank
    PSUM regions across the window; at window end the [64,64] bf16
    weight is applied per block (out_blk = (aggT)^T @ W) and fp32
    results are DMA'd out.
  - Host scatters the 8 output shards back through the dest permutation.
"""

import numpy as np

# ---------------------------------------------------------------- config ---
class CFG:
    def __init__(self, n_nodes, d, n_cores, chunk, nchunks, nblk, window, sub,
                 gmax):
        self.N = n_nodes
        self.D = d
        self.C = n_cores
        self.CHUNK = chunk          # x rows per gather chunk (< 32768 for int16)
        self.NCH = nchunks
        assert chunk * nchunks >= n_nodes
        self.NBLK = nblk            # dest blocks (of 128 rows) per core
        self.CORE_ROWS = 128 * nblk
        assert self.CORE_ROWS * n_cores >= n_nodes
        self.WINDOW = window        # blocks per window
        self.SUB = sub              # slots per DVE/PE subtile
        self.GMAX = gmax            # slots per dma_gather call
        self.windows = [
            (w0, min(w0 + window, nblk)) for w0 in range(0, nblk, window)
        ]


FULL = CFG(n_nodes=100000, d=64, n_cores=8, chunk=25000, nchunks=4,
           nblk=98, window=7, sub=32, gmax=8)


# ------------------------------------------------- dest->bin balancing -----
def balance_bins(deg4, nbins, lanes, cap):
    """Assign dests to bins of `lanes` lanes, balancing per-chunk degree.

    deg4: [ND, NCH] per-dest per-chunk edge counts (ND padded to
    nbins*lanes with zero rows).  Returns bin_of [ND] int32.

    First-fit-decreasing on total degree with a hard per-(bin, chunk)
    capacity of `cap` edges: nearly every bucket then needs exactly
    cap/128 slots, eliminating slot-padding.
    """
    ND, NCH = deg4.shape
    assert ND == nbins * lanes
    tot = deg4.sum(axis=1)
    nz = np.where(tot > 0)[0]
    order = nz[np.argsort(-tot[nz], kind="stable")]
    loads = np.zeros((nbins, NCH), np.int64)
    cnt = np.zeros(nbins, np.int64)
    bin_of = np.full(ND, -1, np.int32)
    kmax = np.argmax(deg4, axis=1)
    for d in order:
        ok = (cnt < lanes) & ((loads + deg4[d]) <= cap).all(axis=1)
        if ok.any():
            score = np.where(ok, loads[:, kmax[d]], 1 << 30)
        else:
            score = np.where(cnt < lanes, (loads + deg4[d]).max(axis=1),
                             1 << 30)
        b = int(np.argmin(score))
        bin_of[d] = b
        loads[b] += deg4[d]
        cnt[b] += 1
    # zero-degree dests fill the remaining lane slots
    zero = np.where(bin_of < 0)[0]
    bin_of[zero] = np.repeat(np.arange(nbins), lanes - cnt)
    return bin_of


# ---------------------------------------------------------- preprocessing ---
def preprocess(x, edge_row, edge_col, edge_val, cfg):
    """Balance dests into 64-lane half-blocks, bucket edges, build arrays."""
    C, NBLK, NCH = cfg.C, cfg.NBLK, cfg.NCH
    NPOS = C * NBLK                 # global 128-dest blocks
    NBIN = NPOS * 2                 # global 64-dest half-blocks
    ND = NPOS * 128

    r = edge_row.astype(np.int64)
    s = edge_col.astype(np.int64)
    k = s // cfg.CHUNK
    lidx = (s % cfg.CHUNK).astype(np.int16)

    # per-dest per-chunk degrees, padded to ND dests
    deg4 = np.bincount(r * NCH + k, minlength=ND * NCH) \
        .reshape(ND, NCH).astype(np.int64)
    bin_of = balance_bins(deg4, NBIN, 64, 256)

    # lanes within each bin (stable order)
    order_d = np.argsort(bin_of, kind="stable")
    lane64_of = np.empty(ND, np.int64)
    lane64_of[order_d] = np.arange(ND) % 64
    block_of = bin_of.astype(np.int64) // 2
    half_of = bin_of.astype(np.int64) % 2
    lane_of = half_of * 64 + lane64_of
    dest_at = np.empty(ND, np.int64)       # (block, lane) -> dest id
    dest_at[block_of * 128 + lane_of] = np.arange(ND)

    gblk = block_of[r]
    h = half_of[r]
    d_lane = lane64_of[r]                  # 0..63 within the half

    # per (bin, chunk) slot needs
    gbin = bin_of[r].astype(np.int64)
    bcounts = np.bincount(gbin * NCH + k, minlength=NBIN * NCH) \
        .reshape(NBIN, NCH)
    bcaps = -(-bcounts // 128)             # [NBIN, NCH]

    # block -> (core, position): align cap vectors across cores per position
    bv = bcaps.reshape(NPOS, 2 * NCH)
    order_g = np.lexsort(tuple(bv[:, j] for j in range(2 * NCH - 1, -1, -1)))
    A = order_g.reshape(NBLK, C)          # A[i, c] = global block id
    pos_c = np.empty(NPOS, np.int64)
    pos_i = np.empty(NPOS, np.int64)
    pos_c[A.reshape(-1)] = np.tile(np.arange(C), NBLK)
    pos_i[A.reshape(-1)] = np.repeat(np.arange(NBLK), C)
    # caps[pos, half, chunk] = max over the 8 cores' blocks at a position
    caps = bv[A].max(axis=1).reshape(NBLK, 2, NCH)
    # every (block, half) needs >= 1 slot so its PSUM region initializes
    for hh in range(2):
        empty = caps[:, hh, :].sum(axis=1) == 0
        caps[empty, hh, 0] = 1

    c = pos_c[r * 0 + gblk]                # pos_c[gblk]
    b = pos_i[gblk]

    # order edges by (core, block, half, chunk, src) -- src-minor for locality
    key = ((((c * NBLK + b) * 2 + h) * NCH + k) * cfg.CHUNK) + lidx
    order = np.argsort(key, kind="stable")
    seg_key = (((c * NBLK + b) * 2 + h) * NCH + k)[order]
    counts = np.bincount(seg_key, minlength=C * NBLK * 2 * NCH) \
        .reshape(C, NBLK, 2, NCH)
    assert (caps[None] >= np.ceil(counts / 128)).all()

    lidx_s = lidx[order]
    val_s = edge_val[order].astype(np.float32)
    d_s = d_lane[order].astype(np.float32)

    starts = np.zeros(C * NBLK * 2 * NCH + 1, dtype=np.int64)
    np.cumsum(counts.reshape(-1), out=starts[1:])

    # plan: per (window, half, chunk) segment, slots grouped by block.
    # Halves are serialized (all of half 0's chunks, then half 1's) so the
    # two accumulation groups sharing one PSUM bank never interleave --
    # accumulation-group state is bank-wide.
    plan = []
    for (w0, w1) in cfg.windows:
        for hh in range(2):
            for kk in range(NCH):
                slot_block = []
                for bb in range(w0, w1):
                    slot_block += [bb] * int(caps[bb, hh, kk])
                plan.append(dict(w0=w0, w1=w1, k=kk, h=hh,
                                 nslots=len(slot_block),
                                 slot_block=slot_block,
                                 slot_half=[hh] * len(slot_block)))

    TOTS = sum(p["nslots"] for p in plan)

    per_core = []
    for cc in range(C):
        idx_mat = np.zeros((128, TOTS * 8), dtype=np.int16)
        val_mat = np.zeros((128, TOTS), dtype=np.float32)
        dst_mat = np.zeros((128, TOTS), dtype=np.float32)
        off = 0
        for p in plan:
            n = p["nslots"]
            if n == 0:
                continue
            kk = p["k"]
            hh = p["h"]
            seg_idx = np.zeros(n * 128, dtype=np.int16)
            seg_val = np.zeros(n * 128, dtype=np.float32)
            seg_dst = np.zeros(n * 128, dtype=np.float32)
            pos = 0
            for bb in range(p["w0"], p["w1"]):
                gi = (((cc * NBLK + bb) * 2) + hh) * NCH + kk
                s0, s1 = starts[gi], starts[gi + 1]
                cnt = s1 - s0
                blk_len = int(caps[bb, hh, kk]) * 128
                seg_idx[pos:pos + cnt] = lidx_s[s0:s1]
                # pad lanes re-read the last real row (val = 0 anyway)
                seg_idx[pos + cnt:pos + blk_len] = \
                    seg_idx[pos + cnt - 1] if cnt > 0 else 0
                seg_val[pos:pos + cnt] = val_s[s0:s1]
                seg_dst[pos:pos + cnt] = d_s[s0:s1]
                pos += blk_len
            assert pos == n * 128
            # gather idx wrap: stream pos j -> (partition j%16, col j//16),
            # replicated into the 8 groups of 16 partitions
            iw = seg_idx.reshape(n * 8, 16).T          # [16, n*8]
            idx_mat[:, off * 8:(off + n) * 8] = np.tile(iw, (8, 1))
            # val/dst wrap: pos j -> (partition j%128, slot j//128)
            val_mat[:, off:off + n] = seg_val.reshape(n, 128).T
            dst_mat[:, off:off + n] = seg_dst.reshape(n, 128).T
            off += n
        per_core.append(dict(idx=idx_mat, val=val_mat, dst=dst_mat))

    return caps, plan, per_core, TOTS, A, dest_at


# ---------------------------------------------------------------- kernel ---
def build_bass(cfg, caps, plan, TOTS):
    import concourse.bacc as bacc
    import concourse.bass as bass
    import concourse.mybir as mybir
    import concourse.tile as tile
    from concourse import library_config
    from concourse._compat import get_trn_type

    f32 = mybir.dt.float32
    bf16 = mybir.dt.bfloat16
    i16 = mybir.dt.int16
    D, NCH = cfg.D, cfg.NCH

    nc = bacc.Bacc(get_trn_type() or "TRN2", target_bir_lowering=False,
                   debug=False, num_swdge_queues=4)
    x_hbm = nc.dram_tensor("x", [cfg.CHUNK * NCH, D], f32,
                           kind="ExternalInput")
    w_hbm = nc.dram_tensor("w", [D, D], bf16, kind="ExternalInput")
    iota_hbm = nc.dram_tensor("iota", [128, 64], bf16, kind="ExternalInput")
    idx_hbm = nc.dram_tensor("idx", [128, TOTS * 8], i16,
                             kind="ExternalInput")
    val_hbm = nc.dram_tensor("val", [128, TOTS], f32, kind="ExternalInput")
    dst_hbm = nc.dram_tensor("dst", [128, TOTS], bf16, kind="ExternalInput")
    out_hbm = nc.dram_tensor("out", [cfg.CORE_ROWS, D], f32,
                             kind="ExternalOutput")

    # (block, half) -> first/last (plan index, slot) for start/stop flags
    first_slot = {}
    last_slot = {}
    for pi, p in enumerate(plan):
        for s, (bb, hh) in enumerate(zip(p["slot_block"], p["slot_half"])):
            key = (int(bb), int(hh))
            if key not in first_slot:
                first_slot[key] = (pi, s)
            last_slot[key] = (pi, s)

    with tile.TileContext(nc) as tc:
        with (
            tc.tile_pool(name="const", bufs=1) as constp,
            tc.tile_pool(name="idxp", bufs=5) as idxp,
            tc.tile_pool(name="valp", bufs=5) as valp,
            tc.tile_pool(name="dstp", bufs=5) as dstp,
            tc.tile_pool(name="gp", bufs=6) as gp,
            tc.tile_pool(name="gvp", bufs=6) as gvp,
            tc.tile_pool(name="sp", bufs=6) as sp,
            tc.tile_pool(name="aggsb", bufs=4) as aggsbp,
            tc.tile_pool(name="stg", bufs=2) as stgp,
            tc.tile_pool(name="aggps", bufs=cfg.WINDOW,
                         space=bass.MemorySpace.PSUM) as aggpsp,
            tc.tile_pool(name="out2ps", bufs=1,
                         space=bass.MemorySpace.PSUM) as out2psp,
        ):
            nc.gpsimd.load_library(library_config.mlp)
            iota_sb = constp.tile([128, 64], bf16, tag="iota")
            w_sb = constp.tile([D, D], bf16, tag="w")
            nc.sync.dma_start(iota_sb[:], iota_hbm[:])
            nc.sync.dma_start(w_sb[:], w_hbm[:])

            nslots_max = max(p["nslots"] for p in plan)
            whalf_max = max(
                sum(plan[b + j]["nslots"] for j in range(NCH))
                for b in range(0, len(plan), NCH))
            gq = 0  # SWDGE queue round-robin counter

            # touch every g buffer once so first-use lanes are never
            # uninitialized SBUF (NaN * 0 = NaN would poison the PSUM)
            for _ in range(6):
                gz = gp.tile([128, nslots_max, D], f32, tag="g")
                nc.vector.memset(gz[:], 0.0)

            for wi, (w0, w1) in enumerate(cfg.windows):
                nb = w1 - w0
                aggps = [aggpsp.tile([64, 128], f32, tag="aggps",
                                     name=f"aggps_w{wi}_{i}")
                         for i in range(nb)]

                for hh in range(2):
                    base_pi = (wi * 2 + hh) * NCH
                    woff = sum(q["nslots"] for q in plan[:base_pi])
                    wlen = sum(plan[base_pi + j]["nslots"] for j in range(NCH))
                    if wlen == 0:
                        continue
                    # one stream load per (window, half): fewer sync-queue
                    # DMAs -> less head-blocking of SWDGE drains
                    idx_t = idxp.tile([128, whalf_max * 8], i16, tag="idx")
                    nc.sync.dma_start(idx_t[:, :wlen * 8],
                                      idx_hbm[:, woff * 8:(woff + wlen) * 8])
                    val_t = valp.tile([128, whalf_max], f32, tag="val")
                    nc.sync.dma_start(val_t[:, :wlen],
                                      val_hbm[:, woff:woff + wlen])
                    dst_t = dstp.tile([128, whalf_max], bf16, tag="dst")
                    nc.sync.dma_start(dst_t[:, :wlen],
                                      dst_hbm[:, woff:woff + wlen])

                    for kk in range(NCH):
                        pi = base_pi + kk
                        p = plan[pi]
                        n = p["nslots"]
                        if n == 0:
                            continue
                        loc = sum(q["nslots"] for q in plan[base_pi:pi])

                        g_t = gp.tile([128, nslots_max, D], f32, tag="g")
                        # GMAX=8 slots (1024 descs) fills one SWDGE ring;
                        # round-robin queues
                        for q0 in range(0, n, cfg.GMAX):
                            q1 = min(q0 + cfg.GMAX, n)
                            nq = (q1 - q0) * 128
                            nc.gpsimd.dma_gather(
                                g_t[:, q0:q1, :],
                                x_hbm[kk * cfg.CHUNK:(kk + 1) * cfg.CHUNK, :],
                                idx_t[:, (loc + q0) * 8:(loc + q1) * 8],
                                nq, nq, D, queue_num=gq % 4)
                            gq += 1

                        for s0 in range(0, n, cfg.SUB):
                            s1 = min(s0 + cfg.SUB, n)
                            ns = s1 - s0
                            gv_t = gvp.tile([128, cfg.SUB, D], bf16, tag="gv")
                            nc.vector.tensor_tensor(
                                gv_t[:, :ns, :], g_t[:, s0:s1, :],
                                val_t[:, loc + s0:loc + s1].unsqueeze(2)
                                    .broadcast_to([128, ns, D]),
                                mybir.AluOpType.mult)
                            s_t = sp.tile([128, cfg.SUB, 64], bf16, tag="s")
                            nc.vector.tensor_tensor(
                                s_t[:, :ns, :],
                                dst_t[:, loc + s0:loc + s1].unsqueeze(2)
                                    .broadcast_to([128, ns, 64]),
                                iota_sb[:, :].unsqueeze(1)
                                    .broadcast_to([128, ns, 64]),
                                mybir.AluOpType.is_equal)
                            for s in range(s0, s1):
                                bb = int(p["slot_block"][s])
                                st = first_slot[(bb, hh)] == (pi, s)
                                sp_ = last_slot[(bb, hh)] == (pi, s)
                                nc.tensor.matmul(
                                    aggps[bb - w0][:, hh * 64:(hh + 1) * 64],
                                    gv_t[:, s - s0, :],
                                    s_t[:, s - s0, :],
                                    start=st, stop=sp_,
                                    skip_group_check=True)

                # ---- flush window: apply W, stage, DMA out
                stg_t = stgp.tile([128, cfg.WINDOW, D], f32, tag="stg")
                out2 = out2psp.tile([128, cfg.WINDOW, D], f32, tag="out2")
                for bi in range(nb):
                    agg_sb = aggsbp.tile([64, 128], bf16, tag="aggsb",
                                         name=f"aggsb_w{wi}_{bi}")
                    nc.scalar.activation(agg_sb[:, :], aggps[bi][:, :],
                                         mybir.ActivationFunctionType.Copy)
                    nc.tensor.matmul(out2[:, bi, :],
                                     agg_sb[:, :], w_sb[:],
                                     start=True, stop=True,
                                     skip_group_check=True)
                nc.scalar.activation(stg_t[:, :nb, :], out2[:, :nb, :],
                                     mybir.ActivationFunctionType.Copy)
                nc.sync.dma_start(
                    out_hbm[w0 * 128:w1 * 128, :]
                    .rearrange("(b p) f -> p b f", p=128),
                    stg_t[:, :nb, :])

    nc.compile()
    return nc


# ------------------------------------------------------------------- run ---
def _to_bf16(a):
    import ml_dtypes
    return a.astype(ml_dtypes.bfloat16)


def run(x, weight, edge_row, edge_col, edge_val, cfg=FULL, trace=False,
        trace_kwargs=None):
    from concourse.bass_utils import run_bass_kernel_spmd

    caps, plan, per_core, TOTS, A, dest_at = preprocess(
        x, edge_row, edge_col, edge_val, cfg)
    nc = build_bass(cfg, caps, plan, TOTS)

    xpad = x
    if cfg.CHUNK * cfg.NCH > cfg.N:
        xpad = np.concatenate(
            [x, np.zeros((cfg.CHUNK * cfg.NCH - cfg.N, cfg.D),
                         dtype=np.float32)], axis=0)
    iota = _to_bf16(np.tile(np.arange(64, dtype=np.float32), (128, 1)))

    in_maps = []
    for cc in range(cfg.C):
        in_maps.append(dict(x=np.ascontiguousarray(xpad),
                            w=_to_bf16(weight),
                            iota=iota,
                            idx=per_core[cc]["idx"],
                            val=per_core[cc]["val"],
                            dst=_to_bf16(per_core[cc]["dst"])))
    kw = {}
    if trace:
        kw = dict(trace=True, trace_kwargs=trace_kwargs or {})
    res = run_bass_kernel_spmd(nc, in_maps, core_ids=list(range(cfg.C)), **kw)
    outs = [r["out"] for r in res.results]
    # un-permute: core c position i lane l holds global dest
    # dest_at[A[i, c]*128 + l]
    full = np.zeros((cfg.C * cfg.NBLK * 128, cfg.D), dtype=np.float32)
    for cc in range(cfg.C):
        dests = dest_at[(A[:, cc][:, None] * 128
                         + np.arange(128)[None, :]).reshape(-1)]
        full[dests] = outs[cc].reshape(-1, cfg.D)
    return full[:cfg.N], res


def kernel(x, weight, edge_row, edge_col, edge_val):
    x = np.asarray(x, dtype=np.float32)
    weight = np.asarray(weight, dtype=np.float32)
    edge_row = np.asarray(edge_row, dtype=np.int32)
    edge_col = np.asarray(edge_col, dtype=np.int32)
    edge_val = np.asarray(edge_val, dtype=np.float32)
    out, _ = run(x, weight, edge_row, edge_col, edge_val, FULL)
    return out
